# revision 15
# baseline (speedup 1.0000x reference)
"""Trainium2 Bass kernel for CausalSelfAttention (q@q^T variant), 8-way
tensor-parallel over heads.

Reference semantics (B=2, T=2048, C=1024, H=16, Dh=64):
    qkv = x @ w_attn + b_attn ; q, k, v = split(qkv)      # k is UNUSED
    att = softmax(causal_mask(q @ q^T / sqrt(Dh)))         # note q@q^T (not k)
    y   = att @ v ; out = y @ w_proj + b_proj

Sharding: core c owns heads {2c, 2c+1}, both batches (tensor parallel).
After attention, an 8-way AllToAll redistributes y from head-sharded to
token-sharded so each core projects (full feature dim) its own 512-row
slice of the flattened [B*T, C] output. b_proj added on host.

All matmuls run in float32r (tf32) at full PE rate.  Scores are computed
directly transposed, sT[key, query], valid because q@q^T is symmetric;
v gets a ones-column so att@v also yields the softmax denominator; causal
masking is an additive -1e30 on the PSUM scores before exp.
"""

import numpy as np

import concourse.bass as bass  # noqa: F401
import concourse.mybir as mybir
import concourse.tile as tile
from concourse import bacc
from concourse.bass_utils import run_bass_kernel_spmd
from concourse.masks import make_identity

f32 = mybir.dt.float32
f32r = mybir.dt.float32r
Act = mybir.ActivationFunctionType

B, T, C, H, DH = 2, 2048, 1024, 16, 64
FT = B * T              # 4096 flat tokens
NCORES = 8
HL = 2                  # heads per core
FL = HL * DH            # 128 local features
NE = C // 128           # 8 contraction chunks
TW = 512                # qkv window
NW = FT // TW           # 8 windows
NTT = FT // 128         # 32 token tiles
TS = FT // NCORES       # 512 output rows per core
SCALE = 1.0 / 8.0       # 1/sqrt(DH)
EXPG = 4                # j-blocks fused per exp strip
VW = 66                 # v slot width: 64 v cols + ones col + even-pad

NEG = -1.0e30

_NC_CACHE = {}

# tuning knobs (read at build time; key the cache)
OPTS = {
    "psS_bufs": 2,
    "psO_bufs": 3,
    "eb_bufs": 3,
    "expg": 4,
    "bcast": "gpsimd",   # or "pe"
}


def build_nc(variant="spmd"):
    key = (variant, tuple(sorted(OPTS.items())))
    if key in _NC_CACHE:
        return _NC_CACHE[key]
    EXPG = OPTS["expg"]
    nc = bacc.Bacc(
        "TRN2",
        target_bir_lowering=False,
        debug=False,
        enable_asserts=True,
        num_devices=NCORES if variant == "spmd" else 1,
    )
    # f32r inputs are host-pre-rounded to tf32 bit patterns
    xT = nc.dram_tensor("xT", [C, FT], f32r, kind="ExternalInput")
    wq = nc.dram_tensor("wq", [C, FL], f32r, kind="ExternalInput")
    wv = nc.dram_tensor("wv", [C, FL], f32r, kind="ExternalInput")
    bq = nc.dram_tensor("bq", [FL, 1], f32, kind="ExternalInput")
    bv = nc.dram_tensor("bv", [FL, 1], f32, kind="ExternalInput")
    wp = nc.dram_tensor("wp", [C, C], f32r, kind="ExternalInput")
    mask = nc.dram_tensor("mask", [128, 512], f32, kind="ExternalInput")
    out = nc.dram_tensor("out", [TS, C], f32, kind="ExternalOutput")

    with tile.TileContext(nc) as tc:
        with tc.tile_pool(name="const", bufs=1) as const:
            wq_sb = const.tile([128, NE, FL], f32r)
            wv_sb = const.tile([128, NE, FL], f32r)
            wp_sb = const.tile([128, NE, C], f32r)
            bq_sb = const.tile([FL, 1], f32)
            bv_sb = const.tile([FL, 1], f32)
            mask_sb = const.tile([128, 512], f32)
            ident = const.tile([128, 128], f32)
            onz = const.tile([128, 2], f32)   # [1.0, 0.0] per partition
            qT_sb = const.tile([128, FT], f32r)          # [f_local, b*T+t]
            v_sb = const.tile([128, NTT, HL * VW], f32r)  # [t_in_tile, tile, h*VW+(d|1|pad)]
            yT_sb = const.tile([64, HL, FT], f32)        # [d, h, b*T+t]
            yTf_sb = const.tile([128, NE, TS], f32)      # post-a2a [f, chunk, t]
            yTr_sb = const.tile([128, NE, TS], f32r)     # rounded copy for matmul

            make_identity(nc, ident)
            nc.vector.memset(onz[:, 0:1], 1.0)
            nc.vector.memset(onz[:, 1:2], 0.0)
            nc.sync.dma_start(out=wq_sb, in_=wq.ap().rearrange("(c p) f -> p c f", p=128))
            nc.sync.dma_start(out=wv_sb, in_=wv.ap().rearrange("(c p) f -> p c f", p=128))
            nc.sync.dma_start(out=wp_sb, in_=wp.ap().rearrange("(c p) f -> p c f", p=128))
            nc.sync.dma_start(out=bq_sb, in_=bq.ap())
            nc.sync.dma_start(out=bv_sb, in_=bv.ap())
            nc.sync.dma_start(out=mask_sb, in_=mask.ap())

            xT_r = xT.ap().rearrange("(c p) t -> p c t", p=128)

            # ---- Phase A: qT / vT production + v transpose ----
            with (
                tc.tile_pool(name="xt", bufs=2) as xpool,
                tc.tile_pool(name="vt", bufs=2) as vtpool,
                tc.tile_pool(name="psA", bufs=2, space="PSUM") as psA,
                tc.tile_pool(name="psT", bufs=2, space="PSUM") as psT,
            ):
                for w in range(OPTS.get('nw', NW)):
                    xt = xpool.tile([128, NE, TW], f32r, tag="xt")
                    nc.sync.dma_start(out=xt, in_=xT_r[:, :, w * TW:(w + 1) * TW])
                    pq = psA.tile([128, TW], f32, tag="pq")
                    for e in range(NE):
                        nc.tensor.matmul(
                            pq, lhsT=wq_sb[:, e, :], rhs=xt[:, e, :],
                            start=(e == 0), stop=(e == NE - 1),
                        )
                    nc.scalar.activation(
                        qT_sb[:, w * TW:(w + 1) * TW], pq, Act.Identity, bias=bq_sb,
                    )
                    pv = psA.tile([128, TW], f32, tag="pv")
                    for e in range(NE):
                        nc.tensor.matmul(
                            pv, lhsT=wv_sb[:, e, :], rhs=xt[:, e, :],
                            start=(e == 0), stop=(e == NE - 1),
                        )
                    vt = vtpool.tile([128, TW], f32, tag="vt")
                    nc.scalar.activation(vt, pv, Act.Identity, bias=bv_sb)
                    for s in range(TW // 128):
                        tt = w * (TW // 128) + s
                        pt = psT.tile([128, 128], f32, tag="pt")
                        nc.tensor.transpose(pt, vt[:, s * 128:(s + 1) * 128], ident)
                        dst = v_sb[:, tt, :].rearrange("p (h x) -> p h x", x=VW)
                        nc.vector.tensor_copy(
                            dst[:, :, 0:64],
                            pt.rearrange("p (h d) -> p h d", d=DH),
                        )
                        nc.vector.tensor_copy(
                            dst[:, :, 64:66],
                            onz.unsqueeze(1).broadcast_to((128, HL, 2)),
                        )

            # ---- Phase B: attention per (batch, head, query-pair) ----
            with (
                tc.tile_pool(name="psS", bufs=OPTS["psS_bufs"], space="PSUM") as psS,
                tc.tile_pool(name="psO", bufs=OPTS["psO_bufs"], space="PSUM") as psO,
                tc.tile_pool(name="eb", bufs=OPTS["eb_bufs"]) as epool,
                tc.tile_pool(name="nrm", bufs=3) as nrm,
                tc.tile_pool(name="psB", bufs=1, space="PSUM") as psB,
            ):
                onesr = None
                if OPTS["bcast"] == "pe":
                    onesr = nrm.tile([1, 64], f32r, tag="onesr")
                    nc.vector.tensor_copy(
                        onesr, onz[0:1, 0:1].broadcast_to((1, 64)),
                    )
                for b in range(B):
                    for h in range(HL):
                        po = h * 64
                        for a in range(OPTS.get('na', T // 256)):
                            nj = 2 * a + 2
                            oT = psO.tile([VW, 256], f32, tag="oT")
                            rq = qT_sb[po:po + 64, b * T + a * 256: b * T + (a + 1) * 256]
                            for g0 in range(0, nj, EXPG):
                                gs = min(EXPG, nj - g0)
                                S = psS.tile([128, EXPG * 256], f32, tag="S")
                                for k in range(gs):
                                    j = g0 + k
                                    nc.tensor.matmul(
                                        S[:, k * 256:(k + 1) * 256],
                                        lhsT=qT_sb[po:po + 64, b * T + j * 128: b * T + (j + 1) * 128],
                                        rhs=rq,
                                        start=True, stop=True,
                                    )
                                    # additive causal mask on the diagonal blocks
                                    if j == 2 * a:
                                        nc.vector.tensor_add(
                                            S[:, k * 256:(k + 1) * 256],
                                            S[:, k * 256:(k + 1) * 256],
                                            mask_sb[:, 0:256],
                                        )
                                    elif j == 2 * a + 1:
                                        nc.vector.tensor_add(
                                            S[:, k * 256:(k + 1) * 256],
                                            S[:, k * 256:(k + 1) * 256],
                                            mask_sb[:, 256:512],
                                        )
                                eb = epool.tile([128, EXPG * 256], f32r, tag="eb")
                                nc.scalar.activation(
                                    eb[:, 0:gs * 256], S[:, 0:gs * 256], Act.Exp, scale=SCALE,
                                )
                                for k in range(gs):
                                    j = g0 + k
                                    nc.tensor.matmul(
                                        oT,
                                        lhsT=v_sb[:, b * (T // 128) + j, h * VW:(h + 1) * VW],
                                        rhs=eb[:, k * 256:(k + 1) * 256],
                                        start=(j == 0), stop=(j == nj - 1),
                                    )
                            rec = nrm.tile([1, 256], f32, tag="rec")
                            nc.vector.reciprocal(rec, oT[64:65, :])
                            if OPTS["bcast"] == "gpsimd":
                                recb = nrm.tile([64, 256], f32, tag="recb")
                                nc.gpsimd.partition_broadcast(recb, rec)
                                nc.vector.tensor_mul(
                                    yT_sb[:, h, b * T + a * 256: b * T + (a + 1) * 256],
                                    oT[0:64, :], recb,
                                )
                            else:
                                recr = nrm.tile([1, 256], f32r, tag="recr")
                                nc.vector.tensor_copy(recr, rec)
                                pb = psB.tile([64, 256], f32, tag="pb")
                                nc.tensor.matmul(pb, lhsT=onesr, rhs=recr,
                                                 start=True, stop=True)
                                nc.vector.tensor_mul(
                                    yT_sb[:, h, b * T + a * 256: b * T + (a + 1) * 256],
                                    oT[0:64, :], pb,
                                )

            # ---- Phase C: AllToAll head->token redistribution ----
            with tc.tile_pool(name="dram", bufs=1, space="DRAM") as dpool:
                a2a_in = dpool.tile([NCORES, HL, 64, TS], f32)
                a2a_out = dpool.tile([NCORES, HL, 64, TS], f32)
                for q in range(NCORES if OPTS.get('do_c', True) else 0):
                    nc.sync.dma_start(
                        out=a2a_in[q].rearrange("h d t -> d h t"),
                        in_=yT_sb[:, :, q * TS:(q + 1) * TS],
                    )
                if not OPTS.get('do_c', True):
                    pass
                elif variant == "spmd":
                    nc.gpsimd.collective_compute(
                        "AllToAll",
                        mybir.AluOpType.bypass,
                        replica_groups=[list(range(NCORES))],
                        ins=[a2a_in.opt()],
                        outs=[a2a_out.opt()],
                    )
                else:  # timeline-estimation variant: plain DRAM copy stand-in
                    nc.sync.dma_start(out=a2a_out[:], in_=a2a_in[:])
                nc.sync.dma_start(
                    out=yTf_sb, in_=a2a_out.rearrange("q h d t -> (h d) q t"),
                )
                # round to tf32 for the projection matmul
                nc.vector.tensor_copy(yTr_sb, yTf_sb)

                # ---- Phase D: output projection on own 512-row slice ----
                with (
                    tc.tile_pool(name="psP", bufs=2, space="PSUM") as psP,
                    tc.tile_pool(name="ob", bufs=2) as outpool,
                ):
                    for ttile in range(TS // 128 if OPTS.get('do_d', True) else 0):
                        ob = outpool.tile([128, C], f32, tag="ob")
                        for cc in range(C // 512):
                            pp = psP.tile([128, 512], f32, tag="pp")
                            for fc in range(NE):
                                nc.tensor.matmul(
                                    pp,
                                    lhsT=yTr_sb[:, fc, ttile * 128:(ttile + 1) * 128],
                                    rhs=wp_sb[:, fc, cc * 512:(cc + 1) * 512],
                                    start=(fc == 0), stop=(fc == NE - 1),
                                )
                            nc.vector.tensor_copy(ob[:, cc * 512:(cc + 1) * 512], pp)
                        nc.sync.dma_start(
                            out=out.ap()[ttile * 128:(ttile + 1) * 128, :], in_=ob,
                        )

    nc.compile()
    _NC_CACHE[key] = nc
    return nc


def _round_tf32(a):
    u = np.ascontiguousarray(a, dtype=np.float32).view(np.uint32)
    r = ((u.astype(np.uint64) + 0x1000) & 0xFFFFE000).astype(np.uint32)
    return r.view(np.float32)


def make_in_maps(input_tokens, w_attn, b_attn, w_proj):
    x = np.ascontiguousarray(np.asarray(input_tokens, dtype=np.float32))
    w_attn = np.asarray(w_attn, dtype=np.float32)
    b_attn = np.asarray(b_attn, dtype=np.float32)
    w_proj = np.asarray(w_proj, dtype=np.float32)

    xT = _round_tf32(np.ascontiguousarray(x.reshape(FT, C).T))  # [C, FT]
    wpr = _round_tf32(np.ascontiguousarray(w_proj))
    # additive causal masks in transposed (key-on-partition) layout:
    # maskA (j == 2a): [tril-valid | all-valid]; maskB (j == 2a+1): [none | tril-valid]
    # invalid = key after query = n > m = strict lower triangle (sT layout)
    tril_bad = np.tril(np.ones((128, 128), dtype=np.float32), k=-1) * NEG
    mask = np.concatenate(
        [tril_bad, np.zeros((128, 128), np.float32),
         np.full((128, 128), NEG, np.float32), tril_bad], axis=1,
    )  # [128, 512]
    in_maps = []
    for c in range(NCORES):
        f0 = c * FL
        in_maps.append({
            "xT": xT,
            "wq": _round_tf32(np.ascontiguousarray(w_attn[:, f0:f0 + FL])),
            "wv": _round_tf32(np.ascontiguousarray(w_attn[:, 2 * C + f0:2 * C + f0 + FL])),
            "bq": np.ascontiguousarray(b_attn[f0:f0 + FL].reshape(FL, 1)),
            "bv": np.ascontiguousarray(b_attn[2 * C + f0:2 * C + f0 + FL].reshape(FL, 1)),
            "wp": wpr,
            "mask": mask,
        })
    return in_maps


def assemble(results, b_proj):
    flat = np.concatenate([results[c]["out"] for c in range(NCORES)], axis=0)
    flat = flat + np.asarray(b_proj, dtype=np.float32)[None, :]
    return flat.reshape(B, T, C)


def kernel(input_tokens, w_attn, b_attn, w_proj, b_proj, _stats=None):
    nc = build_nc()
    in_maps = make_in_maps(input_tokens, w_attn, b_attn, w_proj)
    trace = _stats is not None and _stats.get("trace", False)
    try:
        res = run_bass_kernel_spmd(nc, in_maps, list(range(NCORES)), trace=trace)
    except ModuleNotFoundError:
        # NTFF profile hook unavailable in this environment
        res = run_bass_kernel_spmd(nc, in_maps, list(range(NCORES)), trace=False)
    if _stats is not None:
        _stats["exec_time_ns"] = res.exec_time_ns
        _stats["profile_json"] = res.profile_json
    return assemble(res.results, b_proj)
